# revision 13
# baseline (speedup 1.0000x reference)
"""Distributed Trainium2 Bass kernel for causal multi-head attention with RoPE.

Problem: x[4,2048,1024] f32, wq/wk/wv/wo [1024,1024], token_positions[2048].
out = CausalMHA_RoPE(x) @ wo.T   (16 heads, d_k=64, theta=1e4)

Sharding (8 cores): core c -> batch b=c//2, head-half hh=c%2 (8 heads each).
Per core: QKV projections (bf16 matmuls, inputs pre-transposed on host),
RoPE applied via a host-built cos / signed-sin table pair plus an on-device
partition half-swap (weights are pre-permuted per head so RoPE pairs are
[a-half | b-half] instead of interleaved), flash-style causal attention with
scores in [k, q] orientation (softmax denominators come from a ones-column
appended to V, so the PV matmul emits both O^T and the row sums), pairwise
AllGather of attention outputs, and a column-sliced output projection so the
host reassembly is a pure concat (no host arithmetic on the math path).

Numerical note: softmax is computed without max-subtraction. Inputs are
N(0,1) by spec ("fill": "randn") and scores are ~N(0,1), so exp() stays
within fp32 range comfortably.
"""

import numpy as np

import concourse.bass as bass
import concourse.mybir as mybir
from concourse import bacc
from concourse import library_config
from concourse.tile import TileContext

D_MODEL = 1024
NUM_HEADS = 16
SEQ = 2048
BATCH = 4
THETA = 10000.0
DK = 64
N_CORES = 8
HPC = 8  # heads per core
HD = HPC * DK  # 512, head dims per core

F32 = mybir.dt.float32
BF16 = mybir.dt.bfloat16

REPLICA_GROUPS = [[0, 1], [2, 3], [4, 5], [6, 7]]


# --------------------------------------------------------------------------
# Workaround: this container's walrus rejects >1 semaphore wait on a single
# instruction ("Too many sync wait commands"). Tile's end-of-kernel Drain
# accumulates one wait per active logical proc; split them across multiple
# Drain instructions, one wait each.
# --------------------------------------------------------------------------
_PATCHED = False


def _install_tile_patch():
    global _PATCHED
    if _PATCHED:
        return
    import bass_rust
    import concourse.tile as tile_mod
    from concourse.vector_clock import ScopedClock

    def _patched_drain_and_barrier(self, tick_clock, wait_clock):
        drain_inst = self.nc.sync.drain()
        wait_clock.add_sem_waits(
            drain_inst.ins, ScopedClock({None: tick_clock.global_clock})
        )
        si = drain_inst.ins.sync_info
        if si is not None and si.on_wait and len(si.on_wait) > 1:
            waits = list(si.on_wait)
            drain_inst.ins.sync_info = bass_rust.SyncInfo(
                on_wait=waits[:1], on_update=list(si.on_update)
            )
            for w in waits[1:]:
                extra = self.nc.sync.drain()
                extra.ins.sync_info = bass_rust.SyncInfo(on_wait=[w], on_update=[])

        self.nc.all_engine_barrier()
        assert self.sems is not None
        popped = self.nc._tile_sem_poison_stack.pop()
        assert popped is self._sem_poison
        self.nc.clear_and_free_semaphores(list(self.sems.allocated().values()))
        self.nc.all_engine_barrier()

    tile_mod.TileContext._drain_and_barrier = _patched_drain_and_barrier
    _PATCHED = True


# --------------------------------------------------------------------------
# Device kernel builder (SPMD: one graph, 8 cores)
# --------------------------------------------------------------------------
def build_nc(debug_outputs=False):
    _install_tile_patch()
    nc = bacc.Bacc(None, target_bir_lowering=False)

    xt = nc.declare_dram_parameter("xt", [D_MODEL, SEQ], F32, isOutput=False)
    wqt = nc.declare_dram_parameter("wqt", [D_MODEL, HD], F32, isOutput=False)
    wkt = nc.declare_dram_parameter("wkt", [D_MODEL, HD], F32, isOutput=False)
    wvt = nc.declare_dram_parameter("wvt", [D_MODEL, HD], F32, isOutput=False)
    wot = nc.declare_dram_parameter("wot", [D_MODEL, HD], F32, isOutput=False)
    cosf = nc.declare_dram_parameter("cosf", [128, SEQ], F32, isOutput=False)
    sinsg = nc.declare_dram_parameter("sinsg", [128, SEQ], F32, isOutput=False)
    maskf = nc.declare_dram_parameter("maskf", [128, 128], F32, isOutput=False)
    out = nc.declare_dram_parameter("out", [SEQ, HD], F32, isOutput=True)
    if debug_outputs:
        dbg_qt = nc.declare_dram_parameter("dbg_qt", [128, 4, SEQ], F32, isOutput=True)
        dbg_kt = nc.declare_dram_parameter("dbg_kt", [128, 4, SEQ], F32, isOutput=True)
        dbg_v = nc.declare_dram_parameter("dbg_v", [128, 16, HPC * 65], F32, isOutput=True)
        dbg_st = nc.declare_dram_parameter("dbg_st", [128, 16, 512], F32, isOutput=True)
        dbg_ot = nc.declare_dram_parameter("dbg_ot", [128, 4, SEQ], F32, isOutput=True)
        dbg_abc = {"dbg_qt": dbg_qt, "dbg_kt": dbg_kt, "dbg_v": dbg_v,
                   "dbg_st": dbg_st, "dbg_ot": dbg_ot}

    with TileContext(nc) as tc:
        nc.gpsimd.load_library(library_config.attn)
        with (
            tc.tile_pool(name="dram", bufs=1, space="DRAM") as dramp,
            tc.tile_pool(name="persist", bufs=1) as persist,
        ):
            # Persistent SBUF tensors
            QT = persist.tile([128, 4, SEQ], BF16, name="QT")  # [dk512, s]
            KT = persist.tile([128, 4, SEQ], BF16, name="KT")
            # V rows + per-head ones column: head h at cols 65h..65h+64
            VSB = persist.tile([128, 16, HPC * 65], BF16, name="VSB")
            wot_bf = persist.tile([128, 8, HD], BF16, name="wot_bf")
            mask_bf = persist.tile([128, 128], BF16, name="mask_bf")
            OT_sb = persist.tile([128, 4, SEQ], BF16, name="OT_sb")

            ot_local = dramp.tile([HD, SEQ], BF16, name="ot_local")
            # 2-core groups don't support Shared-scratchpad outputs; plain
            # HBM-to-HBM AllGather works (slower path, overlapped anyway).
            ot_full = dramp.tile([2 * HD, SEQ], BF16, name="ot_full")

            # ------------------ Phase A: load, cast, QKV, RoPE -------------
            with (
                tc.tile_pool(name="phaseA", bufs=1) as pa,
                tc.tile_pool(name="stage", bufs=3) as stage,
                tc.tile_pool(name="psumA", bufs=4, space="PSUM") as psA,
            ):
                xt_bf = pa.tile([128, 8, SEQ], BF16, name="xt_bf")
                wq_bf = pa.tile([128, 8, HD], BF16, name="wq_bf")
                wk_bf = pa.tile([128, 8, HD], BF16, name="wk_bf")
                wv_bf = pa.tile([128, 8, HD], BF16, name="wv_bf")
                cos_bf = pa.tile([128, SEQ], BF16, name="cos_bf")
                sin_bf = pa.tile([128, SEQ], BF16, name="sin_bf")

                for kt in range(8):
                    st = stage.tile([128, SEQ], F32, tag="xstage")
                    nc.sync.dma_start(out=st[:], in_=xt[128 * kt : 128 * kt + 128, :])
                    nc.vector.tensor_copy(out=xt_bf[:, kt, :], in_=st[:])

                for wsrc, wdst in (
                    (wqt, wq_bf),
                    (wkt, wk_bf),
                    (wvt, wv_bf),
                    (wot, wot_bf),
                ):
                    for kt in range(8):
                        stw = stage.tile([128, HD], F32, tag="wstage")
                        nc.sync.dma_start(
                            out=stw[:], in_=wsrc[128 * kt : 128 * kt + 128, :]
                        )
                        nc.scalar.copy(out=wdst[:, kt, :], in_=stw[:])

                stc = stage.tile([128, SEQ], F32, tag="xstage")
                nc.sync.dma_start(out=stc[:], in_=cosf[:, :])
                nc.vector.tensor_copy(out=cos_bf[:], in_=stc[:])
                sts = stage.tile([128, SEQ], F32, tag="xstage")
                nc.sync.dma_start(out=sts[:], in_=sinsg[:, :])
                nc.vector.tensor_copy(out=sin_bf[:], in_=sts[:])

                stm = stage.tile([128, 128], F32, tag="mstage")
                nc.sync.dma_start(out=stm[:], in_=maskf[:, :])
                nc.vector.tensor_copy(out=mask_bf[:], in_=stm[:])

                # Q/K projections: dst[dk, s] = w_slice @ x.T
                for wbf, dst in ((wq_bf, QT), (wk_bf, KT)):
                    for m in range(4):
                        for j in range(4):
                            ps = psA.tile([128, 512], F32, tag="psA")
                            for kt in range(8):
                                nc.tensor.matmul(
                                    ps[:],
                                    lhsT=wbf[:, kt, 128 * m : 128 * m + 128],
                                    rhs=xt_bf[:, kt, 512 * j : 512 * j + 512],
                                    start=(kt == 0),
                                    stop=(kt == 7),
                                )
                            nc.scalar.copy(
                                out=dst[:, m, 512 * j : 512 * j + 512], in_=ps[:]
                            )

                # V projection: V[s, dk] = x @ wv_slice.T, interleaved per head
                for stile in range(16):
                    ps = psA.tile([128, 512], F32, tag="psA")
                    for kt in range(8):
                        nc.tensor.matmul(
                            ps[:],
                            lhsT=xt_bf[:, kt, 128 * stile : 128 * stile + 128],
                            rhs=wv_bf[:, kt, :],
                            start=(kt == 0),
                            stop=(kt == 7),
                        )
                    for h in range(HPC):
                        nc.vector.tensor_copy(
                            out=VSB[:, stile, 65 * h : 65 * h + 64],
                            in_=ps[:, 64 * h : 64 * h + 64],
                        )
                for h in range(HPC):
                    nc.vector.memset(VSB[:, :, 65 * h + 64], 1.0)

                # RoPE on QT, KT: T' = T*cos + swap_halves(T)*signed_sin
                for T in (QT, KT):
                    tsw = stage.tile([128, 4, SEQ], BF16, tag="tswap", bufs=2)
                    for d, s in ((0, 32), (32, 0), (64, 96), (96, 64)):
                        nc.vector.tensor_copy(
                            out=tsw[d : d + 32], in_=T[s : s + 32]
                        )
                    nc.vector.tensor_tensor(
                        out=tsw[:],
                        in0=tsw[:],
                        in1=sin_bf[:, None, :].to_broadcast([128, 4, SEQ]),
                        op=mybir.AluOpType.mult,
                    )
                    nc.vector.tensor_tensor(
                        out=T[:],
                        in0=T[:],
                        in1=cos_bf[:, None, :].to_broadcast([128, 4, SEQ]),
                        op=mybir.AluOpType.mult,
                    )
                    nc.vector.tensor_tensor(
                        out=T[:], in0=T[:], in1=tsw[:], op=mybir.AluOpType.add
                    )

            # ------------------ Phase B: attention per head ----------------
            if debug_outputs:
                with tc.tile_pool(name="dbgst", bufs=2) as dbgpool:
                    for nm, t in (("dbg_qt", QT), ("dbg_kt", KT), ("dbg_v", VSB)):
                        nch = t.shape[1]
                        for ch in range(nch):
                            dt32 = dbgpool.tile([128, t.shape[2]], F32, tag="dbg32")
                            nc.vector.tensor_copy(out=dt32[:], in_=t[:, ch, :])
                            nc.sync.dma_start(out=dbg_abc[nm][:, ch, :], in_=dt32[:])
            with (
                tc.tile_pool(name="phB", bufs=2) as pb,
                tc.tile_pool(name="psum_st", bufs=1, space="PSUM") as ps_st,
                tc.tile_pool(name="psum_o", bufs=2, space="PSUM") as ps_o,
            ):
                for h in range(HPC):
                    pbase = 64 * (h % 2)
                    mt = h // 2
                    for j in range(4):  # q blocks of 512
                        ST = pb.tile([128, 16, 512], BF16, tag="ST")
                        for g in range(j + 1):  # kt groups of 4
                            psg = ps_st.tile([128, 4, 512], F32, tag="stg")
                            for d in range(4):
                                kt = 4 * g + d
                                nc.tensor.matmul(
                                    psg[:, d, :],
                                    lhsT=KT[
                                        pbase : pbase + 64,
                                        mt,
                                        128 * kt : 128 * kt + 128,
                                    ],
                                    rhs=QT[
                                        pbase : pbase + 64,
                                        mt,
                                        512 * j : 512 * j + 512,
                                    ],
                                    start=True,
                                    stop=True,
                                )
                            nc.scalar.activation(
                                out=ST[:, 4 * g : 4 * g + 4, :],
                                in_=psg[:],
                                func=mybir.ActivationFunctionType.Exp,
                            )
                        # causal mask on the 4 diagonal subblocks (group g=j),
                        # plus zero the fully-masked columns left of each
                        for d in range(4):
                            kt = 4 * j + d
                            nc.vector.tensor_tensor(
                                out=ST[:, kt, 128 * d : 128 * d + 128],
                                in0=ST[:, kt, 128 * d : 128 * d + 128],
                                in1=mask_bf[:],
                                op=mybir.AluOpType.mult,
                            )
                            if d > 0:
                                nc.vector.memset(ST[:, kt, 0 : 128 * d], 0.0)
                        # PV with ones column: rows 0-63 = O^T, row 64 = sums
                        po = ps_o.tile([128, 512], F32, tag="po")
                        nkt = 4 * (j + 1)
                        for kt in range(nkt):
                            nc.tensor.matmul(
                                po[0:65, :],
                                lhsT=VSB[:, kt, 65 * h : 65 * h + 65],
                                rhs=ST[:, kt, :],
                                start=(kt == 0),
                                stop=(kt == nkt - 1),
                            )
                        if debug_outputs and h == 0 and j == 3:
                            for dkt in range(16):
                                dst32 = pb.tile([128, 512], F32, tag="dbgch")
                                nc.vector.tensor_copy(out=dst32[:], in_=ST[:, dkt, :])
                                nc.sync.dma_start(out=dbg_abc["dbg_st"][:, dkt, :], in_=dst32[:])
                        # custom DVE ops (reciprocal_approx_fast) drop the
                        # input AP's base_partition on HW: stage the sums row
                        # (psum partition 64) to a base-0 SBUF tile first.
                        s1 = pb.tile([1, 512], F32, tag="s1")
                        nc.vector.tensor_copy(out=s1[:], in_=po[64:65, :])
                        rec = pb.tile([1, 512], F32, tag="rec")
                        nc.vector.reciprocal_approx_fast(out=rec[:], in_=s1[:])
                        rec64 = pb.tile([64, 512], F32, tag="rec64")
                        nc.gpsimd.partition_broadcast(rec64[:], rec[:])
                        nc.vector.tensor_tensor(
                            out=OT_sb[pbase : pbase + 64, mt, 512 * j : 512 * j + 512],
                            in0=po[0:64, :],
                            in1=rec64[:],
                            op=mybir.AluOpType.mult,
                        )

                nc.sync.dma_start(
                    out=ot_local.rearrange("(mt p) s -> p mt s", p=128),
                    in_=OT_sb[:],
                )
                if debug_outputs:
                    for dmt in range(4):
                        for dj in range(4):
                            dot32 = pb.tile([128, 512], F32, tag="dbgch")
                            nc.vector.tensor_copy(out=dot32[:], in_=OT_sb[:, dmt, 512*dj:512*dj+512])
                            nc.sync.dma_start(out=dbg_abc["dbg_ot"][:, dmt, 512*dj:512*dj+512], in_=dot32[:])

            # ------------------ AllGather over the batch pair --------------
            nc.gpsimd.collective_compute(
                "AllGather",
                mybir.AluOpType.bypass,
                replica_groups=REPLICA_GROUPS,
                ins=[ot_local.opt()],
                outs=[ot_full.opt()],
            )

            # ------------------ Phase C: output projection -----------------
            with (
                tc.tile_pool(name="phC", bufs=1) as pc,
                tc.tile_pool(name="stageC", bufs=3) as stc_pool,
                tc.tile_pool(name="psumC", bufs=4, space="PSUM") as psC,
            ):
                OTF = pc.tile([128, 8, SEQ], BF16, name="OTF")
                nc.sync.dma_start(
                    out=OTF[:], in_=ot_full.rearrange("(kt p) s -> p kt s", p=128)
                )
                for stile in range(16):
                    ps = psC.tile([128, 512], F32, tag="psC")
                    for kt in range(8):
                        nc.tensor.matmul(
                            ps[:],
                            lhsT=OTF[:, kt, 128 * stile : 128 * stile + 128],
                            rhs=wot_bf[:, kt, :],
                            start=(kt == 0),
                            stop=(kt == 7),
                        )
                    ost = stc_pool.tile([128, 512], F32, tag="ostage")
                    nc.vector.tensor_copy(out=ost[:], in_=ps[:])
                    nc.sync.dma_start(
                        out=out[128 * stile : 128 * stile + 128, :], in_=ost[:]
                    )

    nc.finalize()
    return nc


# --------------------------------------------------------------------------
# Host-side sharding / table prep
# --------------------------------------------------------------------------
def _rope_perm():
    """Per-head row permutation: interleaved (x0,x1) pairs -> [a-half|b-half]."""
    perm = []
    for h in range(NUM_HEADS):
        base = h * DK
        perm.extend(base + 2 * j for j in range(DK // 2))
        perm.extend(base + 2 * j + 1 for j in range(DK // 2))
    return np.asarray(perm)


def make_in_maps(x, wq, wk, wv, wo, token_positions):
    x = np.asarray(x, dtype=np.float32)
    wq = np.asarray(wq, dtype=np.float32)
    wk = np.asarray(wk, dtype=np.float32)
    wv = np.asarray(wv, dtype=np.float32)
    wo = np.asarray(wo, dtype=np.float32)
    pos = np.asarray(token_positions).astype(np.float32)

    perm = _rope_perm()
    wq_p = wq[perm] * (1.0 / np.sqrt(DK))  # fold softmax scale into wq
    wk_p = wk[perm]

    # RoPE tables in the permuted layout: row p of a 128-row tile covers two
    # heads; freq index = p % 32, a-half rows are p%64<32.
    inv_freq = 1.0 / (THETA ** (np.arange(0, DK, 2, dtype=np.float32) / DK))
    ang = pos[None, :] * inv_freq[:, None]  # [32, S]
    cos32 = np.cos(ang)
    sin32 = np.sin(ang)
    cosf = np.tile(cos32, (4, 1)).astype(np.float32)  # [128, S]
    sinsg = np.concatenate([-sin32, sin32, -sin32, sin32], axis=0).astype(np.float32)

    # causal mask for diagonal 128x128 subblocks in [k, q] layout: keep q>=k
    i = np.arange(128)
    maskf = (i[None, :] >= i[:, None]).astype(np.float32)  # mask[p, f] = f>=p

    in_maps = []
    for c in range(N_CORES):
        b, hh = c // 2, c % 2
        rows = slice(HD * hh, HD * hh + HD)
        in_maps.append(
            {
                "xt": np.ascontiguousarray(x[b].T),
                "wqt": np.ascontiguousarray(wq_p[rows].T),
                "wkt": np.ascontiguousarray(wk_p[rows].T),
                "wvt": np.ascontiguousarray(wv[rows].T),
                "wot": np.ascontiguousarray(wo[rows].T),
                "cosf": cosf,
                "sinsg": sinsg,
                "maskf": maskf,
            }
        )
    return in_maps


def assemble_output(results):
    full = np.empty((BATCH, SEQ, D_MODEL), dtype=np.float32)
    for c in range(N_CORES):
        b, hh = c // 2, c % 2
        full[b, :, HD * hh : HD * hh + HD] = results[c]["out"]
    return full


_NC_CACHE = None


def kernel(x, wq, wk, wv, wo, token_positions):
    global _NC_CACHE
    from concourse.bass_utils import run_bass_kernel_spmd

    if _NC_CACHE is None:
        _NC_CACHE = build_nc()
    in_maps = make_in_maps(x, wq, wk, wv, wo, token_positions)
    res = run_bass_kernel_spmd(_NC_CACHE, in_maps, list(range(N_CORES)))
    return assemble_output(res.results)


# revision 14
# speedup vs baseline: 6065.4390x; 6065.4390x over previous
"""Distributed Trainium2 Bass kernel for causal multi-head attention with RoPE.

Problem: x[4,2048,1024] f32, wq/wk/wv/wo [1024,1024], token_positions[2048].
out = CausalMHA_RoPE(x) @ wo.T   (16 heads, d_k=64, theta=1e4)

Sharding (8 cores): core c -> batch b=c//2, head-half hh=c%2 (8 heads each).
Per core: QKV projections (bf16 matmuls, inputs pre-transposed on host),
RoPE applied via a host-built cos / signed-sin table pair plus an on-device
partition half-swap (weights are pre-permuted per head so RoPE pairs are
[a-half | b-half] instead of interleaved), flash-style causal attention with
scores in [k, q] orientation (softmax denominators come from a ones-column
appended to V, so the PV matmul emits both O^T and the row sums), pairwise
AllGather of attention outputs, and a column-sliced output projection so the
host reassembly is a pure concat (no host arithmetic on the math path).

Numerical note: softmax is computed without max-subtraction. Inputs are
N(0,1) by spec ("fill": "randn") and scores are ~N(0,1), so exp() stays
within fp32 range comfortably.
"""

import numpy as np

import concourse.bass as bass
import concourse.mybir as mybir
from concourse import bacc
from concourse import library_config
from concourse.tile import TileContext

D_MODEL = 1024
NUM_HEADS = 16
SEQ = 2048
BATCH = 4
THETA = 10000.0
DK = 64
N_CORES = 8
HPC = 8  # heads per core
HD = HPC * DK  # 512, head dims per core

F32 = mybir.dt.float32
BF16 = mybir.dt.bfloat16

REPLICA_GROUPS = [[0, 1], [2, 3], [4, 5], [6, 7]]


# --------------------------------------------------------------------------
# Workaround: this container's walrus rejects >1 semaphore wait on a single
# instruction ("Too many sync wait commands"). Tile's end-of-kernel Drain
# accumulates one wait per active logical proc; split them across multiple
# Drain instructions, one wait each.
# --------------------------------------------------------------------------
_PATCHED = False


def _install_tile_patch():
    global _PATCHED
    if _PATCHED:
        return
    import bass_rust
    import concourse.tile as tile_mod
    from concourse.vector_clock import ScopedClock

    def _patched_drain_and_barrier(self, tick_clock, wait_clock):
        drain_inst = self.nc.sync.drain()
        wait_clock.add_sem_waits(
            drain_inst.ins, ScopedClock({None: tick_clock.global_clock})
        )
        si = drain_inst.ins.sync_info
        if si is not None and si.on_wait and len(si.on_wait) > 1:
            waits = list(si.on_wait)
            drain_inst.ins.sync_info = bass_rust.SyncInfo(
                on_wait=waits[:1], on_update=list(si.on_update)
            )
            for w in waits[1:]:
                extra = self.nc.sync.drain()
                extra.ins.sync_info = bass_rust.SyncInfo(on_wait=[w], on_update=[])

        self.nc.all_engine_barrier()
        assert self.sems is not None
        popped = self.nc._tile_sem_poison_stack.pop()
        assert popped is self._sem_poison
        self.nc.clear_and_free_semaphores(list(self.sems.allocated().values()))
        self.nc.all_engine_barrier()

    tile_mod.TileContext._drain_and_barrier = _patched_drain_and_barrier
    _PATCHED = True


# --------------------------------------------------------------------------
# Device kernel builder (SPMD: one graph, 8 cores)
# --------------------------------------------------------------------------
def build_nc(debug_outputs=False, reps=1):
    _install_tile_patch()
    nc = bacc.Bacc(None, target_bir_lowering=False)

    xt = nc.declare_dram_parameter("xt", [D_MODEL, SEQ], F32, isOutput=False)
    wqt = nc.declare_dram_parameter("wqt", [D_MODEL, HD], F32, isOutput=False)
    wkt = nc.declare_dram_parameter("wkt", [D_MODEL, HD], F32, isOutput=False)
    wvt = nc.declare_dram_parameter("wvt", [D_MODEL, HD], F32, isOutput=False)
    wot = nc.declare_dram_parameter("wot", [D_MODEL, HD], F32, isOutput=False)
    cosf = nc.declare_dram_parameter("cosf", [128, SEQ], F32, isOutput=False)
    sinsg = nc.declare_dram_parameter("sinsg", [128, SEQ], F32, isOutput=False)
    maskf = nc.declare_dram_parameter("maskf", [128, 128], F32, isOutput=False)
    out = nc.declare_dram_parameter("out", [SEQ, HD], F32, isOutput=True)
    if debug_outputs:
        dbg_qt = nc.declare_dram_parameter("dbg_qt", [128, 4, SEQ], F32, isOutput=True)
        dbg_kt = nc.declare_dram_parameter("dbg_kt", [128, 4, SEQ], F32, isOutput=True)
        dbg_v = nc.declare_dram_parameter("dbg_v", [128, 16, HPC * 65], F32, isOutput=True)
        dbg_st = nc.declare_dram_parameter("dbg_st", [128, 16, 512], F32, isOutput=True)
        dbg_ot = nc.declare_dram_parameter("dbg_ot", [128, 4, SEQ], F32, isOutput=True)
        dbg_abc = {"dbg_qt": dbg_qt, "dbg_kt": dbg_kt, "dbg_v": dbg_v,
                   "dbg_st": dbg_st, "dbg_ot": dbg_ot}

    with TileContext(nc) as tc:
        nc.gpsimd.load_library(library_config.attn)
        for _rep in range(reps):
            _emit_once(nc, tc, xt, wqt, wkt, wvt, wot, cosf, sinsg, maskf, out,
                       debug_outputs, dbg_abc if debug_outputs else None)

    nc.finalize()
    return nc


def _emit_once(nc, tc, xt, wqt, wkt, wvt, wot, cosf, sinsg, maskf, out,
               debug_outputs, dbg_abc):
    if True:
        with (
            tc.tile_pool(name="dram", bufs=1, space="DRAM") as dramp,
            tc.tile_pool(name="persist", bufs=1) as persist,
        ):
            # Persistent SBUF tensors
            QT = persist.tile([128, 4, SEQ], BF16, name="QT")  # [dk512, s]
            KT = persist.tile([128, 4, SEQ], BF16, name="KT")
            # V rows + per-head ones column: head h at cols 65h..65h+64
            VSB = persist.tile([128, 16, HPC * 65], BF16, name="VSB")
            wot_bf = persist.tile([128, 8, HD], BF16, name="wot_bf")
            mask_bf = persist.tile([128, 128], BF16, name="mask_bf")
            OT_sb = persist.tile([128, 4, SEQ], BF16, name="OT_sb")

            ot_local = dramp.tile([HD, SEQ], BF16, name="ot_local")
            # 2-core groups don't support Shared-scratchpad outputs; plain
            # HBM-to-HBM AllGather works (slower path, overlapped anyway).
            ot_full = dramp.tile([2 * HD, SEQ], BF16, name="ot_full")

            # ------------------ Phase A: load, cast, QKV, RoPE -------------
            with (
                tc.tile_pool(name="phaseA", bufs=1) as pa,
                tc.tile_pool(name="stage", bufs=3) as stage,
                tc.tile_pool(name="psumA", bufs=4, space="PSUM") as psA,
            ):
                xt_bf = pa.tile([128, 8, SEQ], BF16, name="xt_bf")
                wq_bf = pa.tile([128, 8, HD], BF16, name="wq_bf")
                wk_bf = pa.tile([128, 8, HD], BF16, name="wk_bf")
                wv_bf = pa.tile([128, 8, HD], BF16, name="wv_bf")
                cos_bf = pa.tile([128, SEQ], BF16, name="cos_bf")
                sin_bf = pa.tile([128, SEQ], BF16, name="sin_bf")

                for kt in range(8):
                    st = stage.tile([128, SEQ], F32, tag="xstage")
                    nc.sync.dma_start(out=st[:], in_=xt[128 * kt : 128 * kt + 128, :])
                    nc.vector.tensor_copy(out=xt_bf[:, kt, :], in_=st[:])

                for wsrc, wdst in (
                    (wqt, wq_bf),
                    (wkt, wk_bf),
                    (wvt, wv_bf),
                    (wot, wot_bf),
                ):
                    for kt in range(8):
                        stw = stage.tile([128, HD], F32, tag="wstage")
                        nc.sync.dma_start(
                            out=stw[:], in_=wsrc[128 * kt : 128 * kt + 128, :]
                        )
                        nc.scalar.copy(out=wdst[:, kt, :], in_=stw[:])

                stc = stage.tile([128, SEQ], F32, tag="xstage")
                nc.sync.dma_start(out=stc[:], in_=cosf[:, :])
                nc.vector.tensor_copy(out=cos_bf[:], in_=stc[:])
                sts = stage.tile([128, SEQ], F32, tag="xstage")
                nc.sync.dma_start(out=sts[:], in_=sinsg[:, :])
                nc.vector.tensor_copy(out=sin_bf[:], in_=sts[:])

                stm = stage.tile([128, 128], F32, tag="mstage")
                nc.sync.dma_start(out=stm[:], in_=maskf[:, :])
                nc.vector.tensor_copy(out=mask_bf[:], in_=stm[:])

                # Q/K projections: dst[dk, s] = w_slice @ x.T
                for wbf, dst in ((wq_bf, QT), (wk_bf, KT)):
                    for m in range(4):
                        for j in range(4):
                            ps = psA.tile([128, 512], F32, tag="psA")
                            for kt in range(8):
                                nc.tensor.matmul(
                                    ps[:],
                                    lhsT=wbf[:, kt, 128 * m : 128 * m + 128],
                                    rhs=xt_bf[:, kt, 512 * j : 512 * j + 512],
                                    start=(kt == 0),
                                    stop=(kt == 7),
                                )
                            nc.scalar.copy(
                                out=dst[:, m, 512 * j : 512 * j + 512], in_=ps[:]
                            )

                # V projection: V[s, dk] = x @ wv_slice.T, interleaved per head
                for stile in range(16):
                    ps = psA.tile([128, 512], F32, tag="psA")
                    for kt in range(8):
                        nc.tensor.matmul(
                            ps[:],
                            lhsT=xt_bf[:, kt, 128 * stile : 128 * stile + 128],
                            rhs=wv_bf[:, kt, :],
                            start=(kt == 0),
                            stop=(kt == 7),
                        )
                    for h in range(HPC):
                        nc.vector.tensor_copy(
                            out=VSB[:, stile, 65 * h : 65 * h + 64],
                            in_=ps[:, 64 * h : 64 * h + 64],
                        )
                for h in range(HPC):
                    nc.vector.memset(VSB[:, :, 65 * h + 64], 1.0)

                # RoPE on QT, KT: T' = T*cos + swap_halves(T)*signed_sin
                for T in (QT, KT):
                    tsw = stage.tile([128, 4, SEQ], BF16, tag="tswap", bufs=2)
                    for d, s in ((0, 32), (32, 0), (64, 96), (96, 64)):
                        nc.vector.tensor_copy(
                            out=tsw[d : d + 32], in_=T[s : s + 32]
                        )
                    nc.vector.tensor_tensor(
                        out=tsw[:],
                        in0=tsw[:],
                        in1=sin_bf[:, None, :].to_broadcast([128, 4, SEQ]),
                        op=mybir.AluOpType.mult,
                    )
                    nc.vector.tensor_tensor(
                        out=T[:],
                        in0=T[:],
                        in1=cos_bf[:, None, :].to_broadcast([128, 4, SEQ]),
                        op=mybir.AluOpType.mult,
                    )
                    nc.vector.tensor_tensor(
                        out=T[:], in0=T[:], in1=tsw[:], op=mybir.AluOpType.add
                    )

            # ------------------ Phase B: attention per head ----------------
            if debug_outputs:
                with tc.tile_pool(name="dbgst", bufs=2) as dbgpool:
                    for nm, t in (("dbg_qt", QT), ("dbg_kt", KT), ("dbg_v", VSB)):
                        nch = t.shape[1]
                        for ch in range(nch):
                            dt32 = dbgpool.tile([128, t.shape[2]], F32, tag="dbg32")
                            nc.vector.tensor_copy(out=dt32[:], in_=t[:, ch, :])
                            nc.sync.dma_start(out=dbg_abc[nm][:, ch, :], in_=dt32[:])
            with (
                tc.tile_pool(name="phB", bufs=2) as pb,
                tc.tile_pool(name="psum_st", bufs=1, space="PSUM") as ps_st,
                tc.tile_pool(name="psum_o", bufs=2, space="PSUM") as ps_o,
            ):
                for h in range(HPC):
                    pbase = 64 * (h % 2)
                    mt = h // 2
                    for j in range(4):  # q blocks of 512
                        ST = pb.tile([128, 16, 512], BF16, tag="ST")
                        for g in range(j + 1):  # kt groups of 4
                            psg = ps_st.tile([128, 4, 512], F32, tag="stg")
                            for d in range(4):
                                kt = 4 * g + d
                                nc.tensor.matmul(
                                    psg[:, d, :],
                                    lhsT=KT[
                                        pbase : pbase + 64,
                                        mt,
                                        128 * kt : 128 * kt + 128,
                                    ],
                                    rhs=QT[
                                        pbase : pbase + 64,
                                        mt,
                                        512 * j : 512 * j + 512,
                                    ],
                                    start=True,
                                    stop=True,
                                )
                            nc.scalar.activation(
                                out=ST[:, 4 * g : 4 * g + 4, :],
                                in_=psg[:],
                                func=mybir.ActivationFunctionType.Exp,
                            )
                        # causal mask on the 4 diagonal subblocks (group g=j),
                        # plus zero the fully-masked columns left of each
                        for d in range(4):
                            kt = 4 * j + d
                            nc.vector.tensor_tensor(
                                out=ST[:, kt, 128 * d : 128 * d + 128],
                                in0=ST[:, kt, 128 * d : 128 * d + 128],
                                in1=mask_bf[:],
                                op=mybir.AluOpType.mult,
                            )
                            if d > 0:
                                nc.vector.memset(ST[:, kt, 0 : 128 * d], 0.0)
                        # PV with ones column: rows 0-63 = O^T, row 64 = sums
                        po = ps_o.tile([128, 512], F32, tag="po")
                        nkt = 4 * (j + 1)
                        for kt in range(nkt):
                            nc.tensor.matmul(
                                po[0:65, :],
                                lhsT=VSB[:, kt, 65 * h : 65 * h + 65],
                                rhs=ST[:, kt, :],
                                start=(kt == 0),
                                stop=(kt == nkt - 1),
                            )
                        if debug_outputs and h == 0 and j == 3:
                            for dkt in range(16):
                                dst32 = pb.tile([128, 512], F32, tag="dbgch")
                                nc.vector.tensor_copy(out=dst32[:], in_=ST[:, dkt, :])
                                nc.sync.dma_start(out=dbg_abc["dbg_st"][:, dkt, :], in_=dst32[:])
                        # custom DVE ops (reciprocal_approx_fast) drop the
                        # input AP's base_partition on HW: stage the sums row
                        # (psum partition 64) to a base-0 SBUF tile first.
                        s1 = pb.tile([1, 512], F32, tag="s1")
                        nc.vector.tensor_copy(out=s1[:], in_=po[64:65, :])
                        rec = pb.tile([1, 512], F32, tag="rec")
                        nc.vector.reciprocal_approx_fast(out=rec[:], in_=s1[:])
                        rec64 = pb.tile([64, 512], F32, tag="rec64")
                        nc.gpsimd.partition_broadcast(rec64[:], rec[:])
                        nc.vector.tensor_tensor(
                            out=OT_sb[pbase : pbase + 64, mt, 512 * j : 512 * j + 512],
                            in0=po[0:64, :],
                            in1=rec64[:],
                            op=mybir.AluOpType.mult,
                        )

                nc.sync.dma_start(
                    out=ot_local.rearrange("(mt p) s -> p mt s", p=128),
                    in_=OT_sb[:],
                )
                if debug_outputs:
                    for dmt in range(4):
                        for dj in range(4):
                            dot32 = pb.tile([128, 512], F32, tag="dbgch")
                            nc.vector.tensor_copy(out=dot32[:], in_=OT_sb[:, dmt, 512*dj:512*dj+512])
                            nc.sync.dma_start(out=dbg_abc["dbg_ot"][:, dmt, 512*dj:512*dj+512], in_=dot32[:])

            # ------------------ AllGather over the batch pair --------------
            nc.gpsimd.collective_compute(
                "AllGather",
                mybir.AluOpType.bypass,
                replica_groups=REPLICA_GROUPS,
                ins=[ot_local.opt()],
                outs=[ot_full.opt()],
            )

            # ------------------ Phase C: output projection -----------------
            with (
                tc.tile_pool(name="phC", bufs=1) as pc,
                tc.tile_pool(name="stageC", bufs=3) as stc_pool,
                tc.tile_pool(name="psumC", bufs=4, space="PSUM") as psC,
            ):
                OTF = pc.tile([128, 8, SEQ], BF16, name="OTF")
                nc.sync.dma_start(
                    out=OTF[:], in_=ot_full.rearrange("(kt p) s -> p kt s", p=128)
                )
                for stile in range(16):
                    ps = psC.tile([128, 512], F32, tag="psC")
                    for kt in range(8):
                        nc.tensor.matmul(
                            ps[:],
                            lhsT=OTF[:, kt, 128 * stile : 128 * stile + 128],
                            rhs=wot_bf[:, kt, :],
                            start=(kt == 0),
                            stop=(kt == 7),
                        )
                    ost = stc_pool.tile([128, 512], F32, tag="ostage")
                    nc.vector.tensor_copy(out=ost[:], in_=ps[:])
                    nc.sync.dma_start(
                        out=out[128 * stile : 128 * stile + 128, :], in_=ost[:]
                    )


# --------------------------------------------------------------------------
# Host-side sharding / table prep
# --------------------------------------------------------------------------
def _rope_perm():
    """Per-head row permutation: interleaved (x0,x1) pairs -> [a-half|b-half]."""
    perm = []
    for h in range(NUM_HEADS):
        base = h * DK
        perm.extend(base + 2 * j for j in range(DK // 2))
        perm.extend(base + 2 * j + 1 for j in range(DK // 2))
    return np.asarray(perm)


def make_in_maps(x, wq, wk, wv, wo, token_positions):
    x = np.asarray(x, dtype=np.float32)
    wq = np.asarray(wq, dtype=np.float32)
    wk = np.asarray(wk, dtype=np.float32)
    wv = np.asarray(wv, dtype=np.float32)
    wo = np.asarray(wo, dtype=np.float32)
    pos = np.asarray(token_positions).astype(np.float32)

    perm = _rope_perm()
    wq_p = wq[perm] * (1.0 / np.sqrt(DK))  # fold softmax scale into wq
    wk_p = wk[perm]

    # RoPE tables in the permuted layout: row p of a 128-row tile covers two
    # heads; freq index = p % 32, a-half rows are p%64<32.
    inv_freq = 1.0 / (THETA ** (np.arange(0, DK, 2, dtype=np.float32) / DK))
    ang = pos[None, :] * inv_freq[:, None]  # [32, S]
    cos32 = np.cos(ang)
    sin32 = np.sin(ang)
    cosf = np.tile(cos32, (4, 1)).astype(np.float32)  # [128, S]
    sinsg = np.concatenate([-sin32, sin32, -sin32, sin32], axis=0).astype(np.float32)

    # causal mask for diagonal 128x128 subblocks in [k, q] layout: keep q>=k
    i = np.arange(128)
    maskf = (i[None, :] >= i[:, None]).astype(np.float32)  # mask[p, f] = f>=p

    in_maps = []
    for c in range(N_CORES):
        b, hh = c // 2, c % 2
        rows = slice(HD * hh, HD * hh + HD)
        in_maps.append(
            {
                "xt": np.ascontiguousarray(x[b].T),
                "wqt": np.ascontiguousarray(wq_p[rows].T),
                "wkt": np.ascontiguousarray(wk_p[rows].T),
                "wvt": np.ascontiguousarray(wv[rows].T),
                "wot": np.ascontiguousarray(wo[rows].T),
                "cosf": cosf,
                "sinsg": sinsg,
                "maskf": maskf,
            }
        )
    return in_maps


def assemble_output(results):
    full = np.empty((BATCH, SEQ, D_MODEL), dtype=np.float32)
    for c in range(N_CORES):
        b, hh = c // 2, c % 2
        full[b, :, HD * hh : HD * hh + HD] = results[c]["out"]
    return full


_NC_CACHE = None


def kernel(x, wq, wk, wv, wo, token_positions):
    global _NC_CACHE
    from concourse.bass_utils import run_bass_kernel_spmd

    if _NC_CACHE is None:
        _NC_CACHE = build_nc()
    in_maps = make_in_maps(x, wq, wk, wv, wo, token_positions)
    res = run_bass_kernel_spmd(_NC_CACHE, in_maps, list(range(N_CORES)))
    return assemble_output(res.results)
